# revision 17
# baseline (speedup 1.0000x reference)
"""MoE (single shared expert) kernel for 8 trn2 NeuronCores.

Math: the reference's top-2 gating over 64 "experts" feeds a single shared
FFN, and the renormalized top-2 weights sum to s/(s+1e-9) with s >= 1/64,
i.e. 1 up to <= 6.4e-8 relative -- below f32 rounding noise.  The whole
module therefore reduces to:  out = silu(x @ up_w.T) @ down_w.T.

Sharding (8 cores): 2D = 4 token-groups x 2 expert-halves.
Each core (tg, eg) computes the partial
    ytp = ( silu(X[tg] @ up_w[eg].T) @ down_w[:, eg].T ).T      [D, TC]
with X[tg] = 2048 tokens, eg = half of the 2048 expert dims.  The host
sums the two partials of each token group and transposes back.

Default mode is bf16 end-to-end (operands, h, and output partials): the
2e-2 absmax tolerance leaves ~5x margin at bf16's ~4e-3, PE rate is the
same 1 cycle/row as f32r, and every DMA byte count halves.  MOE_MM_DTYPE
selects f32r / f32 for the exact paths.
"""

import os
import sys

import numpy as np

for _p in ("/opt/trn_rl_repo",):
    if os.path.isdir(_p) and _p not in sys.path:
        sys.path.insert(0, _p)

import concourse.bass as bass
import concourse.mybir as mybir
import concourse.tile as tile

F32 = mybir.dt.float32
F32R = mybir.dt.float32r
BF16 = mybir.dt.bfloat16


def _ensure_axon_hooks_shim():
    """bass_utils' trace path imports antenv.axon_hooks, which this image
    lacks; give it a no-op hook module so BASS_TRACE=1 degrades gracefully."""
    import types
    if "antenv.axon_hooks" in sys.modules:
        return
    try:
        import antenv
    except ImportError:
        return
    if hasattr(antenv, "axon_hooks"):
        return
    ah = types.ModuleType("antenv.axon_hooks")
    ah._hook = None
    ah.set_axon_ntff_profile_hook = lambda h: setattr(ah, "_hook", h)
    ah.get_axon_ntff_profile_hook = lambda: ah._hook
    sys.modules["antenv.axon_hooks"] = ah
    antenv.axon_hooks = ah


_ensure_axon_hooks_shim()


def _split_multi_waits(nc):
    """This container's walrus encodes at most ONE sync wait per engine
    instruction ("Too many sync wait commands").  Tile routinely emits
    instructions waiting on several semaphores; hoist the extra waits onto
    single-wait NoOps inserted just before, on the same engine."""
    n = 0
    for f in nc.m.functions:
        for blk in f.blocks:
            insts = blk.instructions
            out = []
            for inst in insts:
                si = inst.sync_info
                waits = list(si.on_wait) if si and si.on_wait else []
                if len(waits) > 1:
                    for w in waits[:-1]:
                        n += 1
                        nop = mybir.InstNoOp(name=f"I-wsplit-{n}", ins=[], outs=[])
                        nop.engine = inst.engine
                        nop.sync_info = mybir.SyncInfo(on_wait=[w], on_update=[])
                        nc.register_instruction(nop)
                        out.append(nop)
                    si.on_wait = [waits[-1]]
                out.append(inst)
            if n:
                insts[:] = out
    return n

def _hoist_early_dmas(nc, n=8):
    """Move the first n wait-free SP DMA pushes from the tile block into the
    main block, before the all-engine barrier: the SP then pushes their
    descriptors ~2us earlier (right after its register init), while the Pool
    engine is still working through const memsets and the barrier dance.
    Safe: their completion semaphores are only waited on inside the tile
    block, and nothing touches the target SBUF tiles before the barrier."""
    blocks = {b.name: b for f in nc.m.functions for b in f.blocks}
    main = blocks.get("main")
    tileb = next((b for name, b in blocks.items()
                  if name.startswith("tile_context") and not name.endswith("_end")),
                 None)
    if main is None or tileb is None:
        return 0
    moved = []
    keep = []
    for inst in tileb.instructions:
        if (len(moved) < n
                and type(inst).__name__ == "InstDMACopy"
                and str(inst.engine) == "EngineType.SP"
                and not (inst.sync_info and inst.sync_info.on_wait)):
            moved.append(inst)
        else:
            keep.append(inst)
    if not moved:
        return 0
    tileb.instructions[:] = keep
    # insert before SP's barrier Drain (first SP non-RegisterMove in main)
    idx = next((i for i, inst in enumerate(main.instructions)
                if str(inst.engine) == "EngineType.SP"
                and type(inst).__name__ != "InstRegisterMove"),
               len(main.instructions))
    main.instructions[idx:idx] = moved
    return len(moved)


# Problem shape (hardcoded per contract)
B, S, D, ED = 4, 2048, 1024, 2048
T = B * S                    # 8192 tokens
TG, EG = 4, 2                # token groups x expert-half groups = 8 cores
TC = T // TG                 # tokens per core      = 2048
EC = ED // EG                # expert dims per core = 1024
TT = 512                     # token tile (matmul free dim)
NTT = TC // TT               # 4 token tiles
NDT = D // 128               # 8 d-tiles (contraction 1 / output rows)
NET = EC // 128              # 8 e-tiles (output rows 1 / contraction 2)

_CACHE = {}
LAST_RESULTS = None          # BassKernelResults of the most recent run


def build_nc(mode: str = "bf16") -> bass.Bass:
    """One-core SPMD program: ytp[D, TC] = (silu(x @ upT) @ dwnT).T partial."""
    mm_dt = {"bf16": BF16, "f32r": F32R, "f32": F32}[mode]
    out_dt = BF16 if mode == "bf16" else F32

    nc = bass.Bass()
    # 3D views of the per-core shards so multi-row-block loads batch into a
    # single DMA_DIRECT descriptor push (the SP pushes descriptors at only
    # ~0.6us each; 32 individual pushes paced the whole input stream).
    xt = nc.dram_tensor("xt", [NTT, NDT, 128, TT], mm_dt, kind="ExternalInput")
    upw = nc.dram_tensor("upw", [NDT, 128, EC], mm_dt, kind="ExternalInput")
    dwn = nc.dram_tensor("dwn", [NET, 128, D], mm_dt, kind="ExternalInput")
    ytp = nc.dram_tensor("ytp", [D, TC], out_dt, kind="ExternalOutput")

    with tile.TileContext(nc) as tc:
        with (
            tc.tile_pool(name="wpool", bufs=1) as wpool,
            tc.tile_pool(name="xpool", bufs=8) as xpool,
            tc.tile_pool(name="hpool", bufs=16) as hpool,
            tc.tile_pool(name="ypool", bufs=6) as ypool,
            tc.tile_pool(name="psum", bufs=8, space="PSUM") as psum,
        ):
            # loop1(0) tiles are individual so each (up[di], x0[di]) DMA
            # pair incrementally unblocks 8 matmuls (~0.4MB granularity,
            # keeps the cold di-major sweep fed with zero PE idle -- any
            # >3.4us idle makes the HAM clock-gate re-throttle, measured).
            # Later tensors are one batched DMA each: a single ~0.6us SP
            # descriptor push per tensor, arriving long before their use.
            up_lo = [wpool.tile([128, EC], mm_dt, tag=f"up{di}", name=f"up{di}")
                     for di in range(NDT)]
            x0_lo = [xpool.tile([128, TT], mm_dt, tag="x", name=f"x0_{di}")
                     for di in range(NDT)]
            xb = {tt: wpool.tile([128, NDT * TT], mm_dt, tag=f"xb{tt}",
                                 name=f"xb{tt}")
                  for tt in (1, 2, 3)}
            dn_big = wpool.tile([128, NET * D], mm_dt, tag="dnb", name="dnb")

            def up_ap(di, eb):
                return up_lo[di][:, eb * 128:(eb + 1) * 128]

            def x_ap(tt, di):
                if tt == 0:
                    return x0_lo[di][:]
                return xb[tt][:, di * TT:(di + 1) * TT]

            def dn_ap(ei, db):
                return dn_big[:, ei * D + db * 128:ei * D + (db + 1) * 128]

            # Warm the PE (HAM clock gate) with dummy matmuls on memset
            # tiles while the initial DMAs stream: the array starts at
            # 1.2GHz and flips to 2.4GHz only after a full ~3.4us activity
            # window of sustained busy-ness; any PE idle before the flip
            # defers it by another window.
            # No memset: the DVE is stuck in the engine preamble until well
            # after the Tensor engine frees up, so a memset dependency makes
            # the warm matmuls start ~2.5us late (measured).  Reading
            # uninitialized SBUF is fine here -- the product is discarded.
            n_warm = int(os.environ.get("MOE_WARM_MM", "7"))
            if n_warm:
                wz = wpool.tile([128, 128], mm_dt, tag="warmw")
                xz = xpool.tile([128, TT], mm_dt, tag="warmx", bufs=1)
                wps = psum.tile([128, TT], F32, tag="ps", name="warm_ps")
                for _ in range(n_warm):
                    nc.tensor.matmul(wps[:], wz[:], xz[:], start=True, stop=True)
                # Writers AFTER the reads: allocates the tiles while leaving
                # the matmuls dependency-free (WAR only orders the memsets).
                nc.vector.memset(wz[:], 0.0)
                nc.vector.memset(xz[:], 0.0)
                wsink = ypool.tile([128, TT], F32, tag="warmy", name="warm_sink")
                nc.vector.tensor_copy(wsink[:], wps[:])

            # All input DMAs up front, in consumption order.  The first four
            # (up[di], x0[di]) pairs arrive one by one and unblock the
            # di-major cold sweep after ~0.4MB; the batched rest needs no
            # further SP pushes, freeing the queue for output DMAs.
            for di in range(NDT):
                nc.sync.dma_start(out=up_lo[di][:], in_=upw[di])
                nc.sync.dma_start(out=x0_lo[di][:], in_=xt[0, di])
            nc.sync.dma_start(
                out=xb[1][:].rearrange("p (a c) -> p a c", a=NDT),
                in_=xt[1].rearrange("a p c -> p a c"),
            )
            nc.sync.dma_start(
                out=dn_big[:].rearrange("p (a c) -> p a c", a=NET),
                in_=dwn[:].rearrange("a p c -> p a c"),
            )
            nc.sync.dma_start(
                out=xb[2][:].rearrange("p (a c) -> p a c", a=NDT),
                in_=xt[2].rearrange("a p c -> p a c"),
            )
            nc.sync.dma_start(
                out=xb[3][:].rearrange("p (a c) -> p a c", a=NDT),
                in_=xt[3].rearrange("a p c -> p a c"),
            )

            hs_all = {}

            def loop1(tt, split_k):
                pss = []
                if split_k:
                    # Fully di-major: MM(di, eb) needs only the (up[di],
                    # x0[di]) pair resident, so the sweep rides the incoming
                    # DMA stream pair by pair with no bulk wait.  Stops at
                    # di==7 stagger eb-by-eb, so the ACT/MUL drain pipelines.
                    pss = [psum.tile([128, TT], F32, tag="ps", name=f"ps1_{tt}_{eb}")
                           for eb in range(NET)]
                    for di in range(NDT):
                        for eb in range(NET):
                            nc.tensor.matmul(
                                pss[eb][:], up_ap(di, eb), x_ap(tt, di),
                                start=(di == 0), stop=(di == NDT - 1),
                            )
                hs = []
                for eb in range(NET):
                    if split_k:
                        ps = pss[eb]
                    else:
                        ps = psum.tile([128, TT], F32, tag="ps",
                                       name=f"ps1_{tt}_{eb}")
                        for di in range(NDT):
                            nc.tensor.matmul(
                                ps[:], up_ap(di, eb), x_ap(tt, di),
                                start=(di == 0), stop=(di == NDT - 1),
                            )
                    sg = hpool.tile([128, TT], F32, tag="sg", bufs=3)
                    nc.scalar.activation(
                        sg[:], ps[:], mybir.ActivationFunctionType.Sigmoid
                    )
                    h = hpool.tile([128, TT], mm_dt, tag="h", bufs=20)
                    nc.vector.tensor_mul(h[:], ps[:], sg[:])
                    hs.append(h)
                hs_all[tt] = hs

            def loop2(tt, ei_major):
                t0 = tt * TT
                hs = hs_all.pop(tt)
                if ei_major:
                    ps2s = [psum.tile([128, TT], F32, tag="ps", name=f"ps2_{tt}_{db}")
                            for db in range(NDT)]
                    for ei in range(NET):
                        for db in range(NDT):
                            nc.tensor.matmul(
                                ps2s[db][:], dn_ap(ei, db), hs[ei][:],
                                start=(ei == 0), stop=(ei == NET - 1),
                            )
                    for db in range(NDT):
                        y = ypool.tile([128, TT], out_dt, tag="y")
                        nc.vector.tensor_copy(y[:], ps2s[db][:])
                        nc.sync.dma_start(
                            out=ytp[db * 128:(db + 1) * 128, t0:t0 + TT],
                            in_=y[:],
                        )
                else:
                    for db in range(NDT):
                        if tt == NTT - 1 and db == NDT - 1:
                            # Last group of the kernel: asymmetric 384/128
                            # column split so the final MM->copy->DMA->
                            # completion chain hangs off only a 32KB piece.
                            dsl = slice(db * 128, (db + 1) * 128)
                            wA = 384
                            psA = psum.tile([128, wA], F32, tag="ps",
                                            name="ps2_last_a")
                            psB = psum.tile([128, TT - wA], F32, tag="ps",
                                            name="ps2_last_b")
                            for ei in range(NET):
                                nc.tensor.matmul(
                                    psA[:], dn_ap(ei, db), hs[ei][:, 0:wA],
                                    start=(ei == 0), stop=(ei == NET - 1),
                                )
                            yA = ypool.tile([128, wA], out_dt, tag="y2", bufs=2)
                            nc.vector.tensor_copy(yA[:], psA[:])
                            nc.sync.dma_start(
                                out=ytp[dsl, t0:t0 + wA], in_=yA[:],
                            )
                            for ei in range(NET):
                                nc.tensor.matmul(
                                    psB[:], dn_ap(ei, db), hs[ei][:, wA:TT],
                                    start=(ei == 0), stop=(ei == NET - 1),
                                )
                            yB = ypool.tile([128, TT - wA], out_dt, tag="y2",
                                            bufs=2)
                            nc.vector.tensor_copy(yB[:], psB[:])
                            nc.sync.dma_start(
                                out=ytp[dsl, t0 + wA:t0 + TT], in_=yB[:],
                            )
                            continue
                        ps2 = psum.tile([128, TT], F32, tag="ps",
                                        name=f"ps2_{tt}_{db}")
                        for ei in range(NET):
                            nc.tensor.matmul(
                                ps2[:], dn_ap(ei, db), hs[ei][:],
                                start=(ei == 0), stop=(ei == NET - 1),
                            )
                        y = ypool.tile([128, TT], out_dt, tag="y")
                        nc.vector.tensor_copy(y[:], ps2[:])
                        nc.sync.dma_start(
                            out=ytp[db * 128:(db + 1) * 128, t0:t0 + TT],
                            in_=y[:],
                        )

            loop1(0, split_k=True)
            loop1(1, split_k=False)
            loop2(0, ei_major=True)
            loop1(2, split_k=False)
            loop2(1, ei_major=False)
            loop1(3, split_k=False)
            loop2(2, ei_major=False)
            loop2(3, ei_major=False)

    _split_multi_waits(nc)
    if os.environ.get("MOE_EARLY_DMA", "1") == "1":
        _hoist_early_dmas(nc, n=int(os.environ.get("MOE_EARLY_DMA_N", "8")))
    nc.finalize()
    return nc


def _get_nc(mode: str) -> bass.Bass:
    if mode not in _CACHE:
        _CACHE[mode] = build_nc(mode)
    return _CACHE[mode]


def kernel(x, gate_w, up_w, down_w):
    global LAST_RESULTS
    from concourse.bass_utils import run_bass_kernel_spmd

    mode = os.environ.get("MOE_MM_DTYPE", "bf16")
    nc = _get_nc(mode)

    if mode == "bf16":
        import ml_dtypes
        host_dt = np.dtype(ml_dtypes.bfloat16)
    else:
        host_dt = np.dtype(np.float32)

    xf = np.ascontiguousarray(np.asarray(x, dtype=np.float32).reshape(T, D))
    up = np.asarray(up_w, dtype=np.float32)
    dn = np.asarray(down_w, dtype=np.float32)

    # [D, TC] -> [NTT, NDT, 128, TT]: every (tt, di) x-tile is a fully
    # contiguous 128KB block, so DMA descriptors hit max efficiency.
    xts = [np.ascontiguousarray(
               xf[tg * TC:(tg + 1) * TC, :].T.reshape(NDT, 128, NTT, TT)
               .transpose(2, 0, 1, 3)).astype(host_dt)
           for tg in range(TG)]
    upts = [np.ascontiguousarray(up[eg * EC:(eg + 1) * EC, :].T).astype(host_dt)
            .reshape(NDT, 128, EC) for eg in range(EG)]
    dnts = [np.ascontiguousarray(dn[:, eg * EC:(eg + 1) * EC].T).astype(host_dt)
            .reshape(NET, 128, D) for eg in range(EG)]

    in_maps = []
    for c in range(8):
        tg, eg = c // EG, c % EG
        in_maps.append({"xt": xts[tg], "upw": upts[eg], "dwn": dnts[eg]})

    res = run_bass_kernel_spmd(nc, in_maps, list(range(8)))
    LAST_RESULTS = res

    out = np.empty((T, D), dtype=np.float32)
    for tg in range(TG):
        part = (np.asarray(res.results[tg * EG]["ytp"], dtype=np.float32)
                + np.asarray(res.results[tg * EG + 1]["ytp"], dtype=np.float32))
        out[tg * TC:(tg + 1) * TC, :] = part.T
    return out.reshape(B, S, D)



# revision 18
# speedup vs baseline: 1.0215x; 1.0215x over previous
"""MoE (single shared expert) kernel for 8 trn2 NeuronCores.

Math: the reference's top-2 gating over 64 "experts" feeds a single shared
FFN, and the renormalized top-2 weights sum to s/(s+1e-9) with s >= 1/64,
i.e. 1 up to <= 6.4e-8 relative -- below f32 rounding noise.  The whole
module therefore reduces to:  out = silu(x @ up_w.T) @ down_w.T.

Sharding (8 cores): 2D = 4 token-groups x 2 expert-halves.
Each core (tg, eg) computes the partial
    ytp = ( silu(X[tg] @ up_w[eg].T) @ down_w[:, eg].T ).T      [D, TC]
with X[tg] = 2048 tokens, eg = half of the 2048 expert dims.  The host
sums the two partials of each token group and transposes back.

Default mode is bf16 end-to-end (operands, h, and output partials): the
2e-2 absmax tolerance leaves ~5x margin at bf16's ~4e-3, PE rate is the
same 1 cycle/row as f32r, and every DMA byte count halves.  MOE_MM_DTYPE
selects f32r / f32 for the exact paths.
"""

import os
import sys

import numpy as np

for _p in ("/opt/trn_rl_repo",):
    if os.path.isdir(_p) and _p not in sys.path:
        sys.path.insert(0, _p)

import concourse.bass as bass
import concourse.mybir as mybir
import concourse.tile as tile

F32 = mybir.dt.float32
F32R = mybir.dt.float32r
BF16 = mybir.dt.bfloat16


def _ensure_axon_hooks_shim():
    """bass_utils' trace path imports antenv.axon_hooks, which this image
    lacks; give it a no-op hook module so BASS_TRACE=1 degrades gracefully."""
    import types
    if "antenv.axon_hooks" in sys.modules:
        return
    try:
        import antenv
    except ImportError:
        return
    if hasattr(antenv, "axon_hooks"):
        return
    ah = types.ModuleType("antenv.axon_hooks")
    ah._hook = None
    ah.set_axon_ntff_profile_hook = lambda h: setattr(ah, "_hook", h)
    ah.get_axon_ntff_profile_hook = lambda: ah._hook
    sys.modules["antenv.axon_hooks"] = ah
    antenv.axon_hooks = ah


_ensure_axon_hooks_shim()


def _split_multi_waits(nc):
    """This container's walrus encodes at most ONE sync wait per engine
    instruction ("Too many sync wait commands").  Tile routinely emits
    instructions waiting on several semaphores; hoist the extra waits onto
    single-wait NoOps inserted just before, on the same engine."""
    n = 0
    for f in nc.m.functions:
        for blk in f.blocks:
            insts = blk.instructions
            out = []
            for inst in insts:
                si = inst.sync_info
                waits = list(si.on_wait) if si and si.on_wait else []
                if len(waits) > 1:
                    for w in waits[:-1]:
                        n += 1
                        nop = mybir.InstNoOp(name=f"I-wsplit-{n}", ins=[], outs=[])
                        nop.engine = inst.engine
                        nop.sync_info = mybir.SyncInfo(on_wait=[w], on_update=[])
                        nc.register_instruction(nop)
                        out.append(nop)
                    si.on_wait = [waits[-1]]
                out.append(inst)
            if n:
                insts[:] = out
    return n

def _hoist_early_dmas(nc, n=8):
    """Move the first n wait-free SP DMA pushes from the tile block into the
    main block, before the all-engine barrier: the SP then pushes their
    descriptors ~2us earlier (right after its register init), while the Pool
    engine is still working through const memsets and the barrier dance.
    Safe: their completion semaphores are only waited on inside the tile
    block, and nothing touches the target SBUF tiles before the barrier."""
    blocks = {b.name: b for f in nc.m.functions for b in f.blocks}
    main = blocks.get("main")
    tileb = next((b for name, b in blocks.items()
                  if name.startswith("tile_context") and not name.endswith("_end")),
                 None)
    if main is None or tileb is None:
        return 0
    moved = []
    keep = []
    for inst in tileb.instructions:
        if (len(moved) < n
                and type(inst).__name__ == "InstDMACopy"
                and str(inst.engine) == "EngineType.SP"
                and not (inst.sync_info and inst.sync_info.on_wait)):
            moved.append(inst)
        else:
            keep.append(inst)
    if not moved:
        return 0
    tileb.instructions[:] = keep
    # insert before SP's barrier Drain (first SP non-RegisterMove in main)
    idx = next((i for i, inst in enumerate(main.instructions)
                if str(inst.engine) == "EngineType.SP"
                and type(inst).__name__ != "InstRegisterMove"),
               len(main.instructions))
    main.instructions[idx:idx] = moved
    return len(moved)


# Problem shape (hardcoded per contract)
B, S, D, ED = 4, 2048, 1024, 2048
T = B * S                    # 8192 tokens
TG, EG = 4, 2                # token groups x expert-half groups = 8 cores
TC = T // TG                 # tokens per core      = 2048
EC = ED // EG                # expert dims per core = 1024
TT = 512                     # token tile (matmul free dim)
NTT = TC // TT               # 4 token tiles
NDT = D // 128               # 8 d-tiles (contraction 1 / output rows)
NET = EC // 128              # 8 e-tiles (output rows 1 / contraction 2)

_CACHE = {}
LAST_RESULTS = None          # BassKernelResults of the most recent run


def build_nc(mode: str = "bf16") -> bass.Bass:
    """One-core SPMD program: ytp[D, TC] = (silu(x @ upT) @ dwnT).T partial."""
    mm_dt = {"bf16": BF16, "f32r": F32R, "f32": F32}[mode]
    out_dt = BF16 if mode == "bf16" else F32

    nc = bass.Bass()
    # 3D views of the per-core shards so multi-row-block loads batch into a
    # single DMA_DIRECT descriptor push (the SP pushes descriptors at only
    # ~0.6us each; 32 individual pushes paced the whole input stream).
    xt = nc.dram_tensor("xt", [NTT, NDT, 128, TT], mm_dt, kind="ExternalInput")
    upw = nc.dram_tensor("upw", [NDT, 128, EC], mm_dt, kind="ExternalInput")
    dwn = nc.dram_tensor("dwn", [NET, 128, D], mm_dt, kind="ExternalInput")
    ytp = nc.dram_tensor("ytp", [D, TC], out_dt, kind="ExternalOutput")

    with tile.TileContext(nc) as tc:
        with (
            tc.tile_pool(name="wpool", bufs=1) as wpool,
            tc.tile_pool(name="xpool", bufs=8) as xpool,
            tc.tile_pool(name="hpool", bufs=16) as hpool,
            tc.tile_pool(name="ypool", bufs=6) as ypool,
            tc.tile_pool(name="psum", bufs=8, space="PSUM") as psum,
        ):
            # loop1(0) tiles are individual so each (up[di], x0[di]) DMA
            # pair incrementally unblocks 8 matmuls (~0.4MB granularity,
            # keeps the cold di-major sweep fed with zero PE idle -- any
            # >3.4us idle makes the HAM clock-gate re-throttle, measured).
            # Later tensors are one batched DMA each: a single ~0.6us SP
            # descriptor push per tensor, arriving long before their use.
            up_lo = [wpool.tile([128, EC], mm_dt, tag=f"up{di}", name=f"up{di}")
                     for di in range(NDT)]
            x0_lo = [xpool.tile([128, TT], mm_dt, tag="x", name=f"x0_{di}")
                     for di in range(NDT)]
            xb = {tt: wpool.tile([128, NDT * TT], mm_dt, tag=f"xb{tt}",
                                 name=f"xb{tt}")
                  for tt in (1, 2, 3)}
            dn_big = wpool.tile([128, NET * D], mm_dt, tag="dnb", name="dnb")

            def up_ap(di, eb):
                return up_lo[di][:, eb * 128:(eb + 1) * 128]

            def x_ap(tt, di):
                if tt == 0:
                    return x0_lo[di][:]
                return xb[tt][:, di * TT:(di + 1) * TT]

            def dn_ap(ei, db):
                return dn_big[:, ei * D + db * 128:ei * D + (db + 1) * 128]

            # Warm the PE (HAM clock gate) with dummy matmuls on memset
            # tiles while the initial DMAs stream: the array starts at
            # 1.2GHz and flips to 2.4GHz only after a full ~3.4us activity
            # window of sustained busy-ness; any PE idle before the flip
            # defers it by another window.
            # No memset: the DVE is stuck in the engine preamble until well
            # after the Tensor engine frees up, so a memset dependency makes
            # the warm matmuls start ~2.5us late (measured).  Reading
            # uninitialized SBUF is fine here -- the product is discarded.
            n_warm = int(os.environ.get("MOE_WARM_MM", "7"))
            if n_warm:
                wz = wpool.tile([128, 128], mm_dt, tag="warmw")
                xz = xpool.tile([128, TT], mm_dt, tag="warmx", bufs=1)
                wps = psum.tile([128, TT], F32, tag="ps", name="warm_ps")
                for _ in range(n_warm):
                    nc.tensor.matmul(wps[:], wz[:], xz[:], start=True, stop=True)
                # Writers AFTER the reads: allocates the tiles while leaving
                # the matmuls dependency-free (WAR only orders the memsets).
                nc.vector.memset(wz[:], 0.0)
                nc.vector.memset(xz[:], 0.0)
                wsink = ypool.tile([128, TT], F32, tag="warmy", name="warm_sink")
                nc.vector.tensor_copy(wsink[:], wps[:])

            # All input DMAs up front, in consumption order.  The first four
            # (up[di], x0[di]) pairs arrive one by one and unblock the
            # di-major cold sweep after ~0.4MB; the batched rest needs no
            # further SP pushes, freeing the queue for output DMAs.
            for di in range(NDT):
                nc.sync.dma_start(out=up_lo[di][:], in_=upw[di])
                nc.sync.dma_start(out=x0_lo[di][:], in_=xt[0, di])
            nc.sync.dma_start(
                out=xb[1][:].rearrange("p (a c) -> p a c", a=NDT),
                in_=xt[1].rearrange("a p c -> p a c"),
            )
            nc.sync.dma_start(
                out=dn_big[:].rearrange("p (a c) -> p a c", a=NET),
                in_=dwn[:].rearrange("a p c -> p a c"),
            )
            nc.sync.dma_start(
                out=xb[2][:].rearrange("p (a c) -> p a c", a=NDT),
                in_=xt[2].rearrange("a p c -> p a c"),
            )
            nc.sync.dma_start(
                out=xb[3][:].rearrange("p (a c) -> p a c", a=NDT),
                in_=xt[3].rearrange("a p c -> p a c"),
            )

            hs_all = {}

            def loop1(tt, split_k):
                pss = []
                if split_k:
                    # Fully di-major: MM(di, eb) needs only the (up[di],
                    # x0[di]) pair resident, so the sweep rides the incoming
                    # DMA stream pair by pair with no bulk wait.  Stops at
                    # di==7 stagger eb-by-eb, so the ACT/MUL drain pipelines.
                    pss = [psum.tile([128, TT], F32, tag="ps", name=f"ps1_{tt}_{eb}")
                           for eb in range(NET)]
                    for di in range(NDT):
                        for eb in range(NET):
                            nc.tensor.matmul(
                                pss[eb][:], up_ap(di, eb), x_ap(tt, di),
                                start=(di == 0), stop=(di == NDT - 1),
                            )
                hs = []
                for eb in range(NET):
                    if split_k:
                        ps = pss[eb]
                    else:
                        ps = psum.tile([128, TT], F32, tag="ps",
                                       name=f"ps1_{tt}_{eb}")
                        for di in range(NDT):
                            nc.tensor.matmul(
                                ps[:], up_ap(di, eb), x_ap(tt, di),
                                start=(di == 0), stop=(di == NDT - 1),
                            )
                    sg = hpool.tile([128, TT], F32, tag="sg", bufs=3)
                    nc.scalar.activation(
                        sg[:], ps[:], mybir.ActivationFunctionType.Sigmoid
                    )
                    h = hpool.tile([128, TT], mm_dt, tag="h", bufs=20)
                    nc.vector.tensor_mul(h[:], ps[:], sg[:])
                    hs.append(h)
                hs_all[tt] = hs

            def loop2(tt, ei_major):
                t0 = tt * TT
                hs = hs_all.pop(tt)
                if ei_major:
                    ps2s = [psum.tile([128, TT], F32, tag="ps", name=f"ps2_{tt}_{db}")
                            for db in range(NDT)]
                    for ei in range(NET):
                        for db in range(NDT):
                            nc.tensor.matmul(
                                ps2s[db][:], dn_ap(ei, db), hs[ei][:],
                                start=(ei == 0), stop=(ei == NET - 1),
                            )
                    for db in range(NDT):
                        y = ypool.tile([128, TT], out_dt, tag="y")
                        nc.vector.tensor_copy(y[:], ps2s[db][:])
                        nc.sync.dma_start(
                            out=ytp[db * 128:(db + 1) * 128, t0:t0 + TT],
                            in_=y[:],
                        )
                else:
                    for db in range(NDT):
                        if tt == NTT - 1 and db == NDT - 1:
                            # Last group of the kernel: asymmetric 384/128
                            # column split so the final MM->copy->DMA->
                            # completion chain hangs off only a 32KB piece.
                            dsl = slice(db * 128, (db + 1) * 128)
                            wA = 384
                            psA = psum.tile([128, wA], F32, tag="ps",
                                            name="ps2_last_a")
                            psB = psum.tile([128, TT - wA], F32, tag="ps",
                                            name="ps2_last_b")
                            for ei in range(NET):
                                nc.tensor.matmul(
                                    psA[:], dn_ap(ei, db), hs[ei][:, 0:wA],
                                    start=(ei == 0), stop=(ei == NET - 1),
                                )
                            yA = ypool.tile([128, wA], out_dt, tag="y2", bufs=2)
                            nc.vector.tensor_copy(yA[:], psA[:])
                            nc.sync.dma_start(
                                out=ytp[dsl, t0:t0 + wA], in_=yA[:],
                            )
                            for ei in range(NET):
                                nc.tensor.matmul(
                                    psB[:], dn_ap(ei, db), hs[ei][:, wA:TT],
                                    start=(ei == 0), stop=(ei == NET - 1),
                                )
                            yB = ypool.tile([128, TT - wA], out_dt, tag="y2",
                                            bufs=2)
                            nc.vector.tensor_copy(yB[:], psB[:])
                            nc.sync.dma_start(
                                out=ytp[dsl, t0 + wA:t0 + TT], in_=yB[:],
                            )
                            continue
                        ps2 = psum.tile([128, TT], F32, tag="ps",
                                        name=f"ps2_{tt}_{db}")
                        for ei in range(NET):
                            nc.tensor.matmul(
                                ps2[:], dn_ap(ei, db), hs[ei][:],
                                start=(ei == 0), stop=(ei == NET - 1),
                            )
                        y = ypool.tile([128, TT], out_dt, tag="y")
                        nc.vector.tensor_copy(y[:], ps2[:])
                        nc.sync.dma_start(
                            out=ytp[db * 128:(db + 1) * 128, t0:t0 + TT],
                            in_=y[:],
                        )

            loop1(0, split_k=True)
            loop1(1, split_k=False)
            loop2(0, ei_major=True)
            loop1(2, split_k=False)
            loop2(1, ei_major=False)
            loop1(3, split_k=False)
            loop2(2, ei_major=False)
            loop2(3, ei_major=False)

    _split_multi_waits(nc)
    if os.environ.get("MOE_EARLY_DMA", "0") == "1":
        _hoist_early_dmas(nc, n=int(os.environ.get("MOE_EARLY_DMA_N", "8")))
    nc.finalize()
    return nc


def _get_nc(mode: str) -> bass.Bass:
    if mode not in _CACHE:
        _CACHE[mode] = build_nc(mode)
    return _CACHE[mode]


def kernel(x, gate_w, up_w, down_w):
    global LAST_RESULTS
    from concourse.bass_utils import run_bass_kernel_spmd

    mode = os.environ.get("MOE_MM_DTYPE", "bf16")
    nc = _get_nc(mode)

    if mode == "bf16":
        import ml_dtypes
        host_dt = np.dtype(ml_dtypes.bfloat16)
    else:
        host_dt = np.dtype(np.float32)

    xf = np.ascontiguousarray(np.asarray(x, dtype=np.float32).reshape(T, D))
    up = np.asarray(up_w, dtype=np.float32)
    dn = np.asarray(down_w, dtype=np.float32)

    # [D, TC] -> [NTT, NDT, 128, TT]: every (tt, di) x-tile is a fully
    # contiguous 128KB block, so DMA descriptors hit max efficiency.
    xts = [np.ascontiguousarray(
               xf[tg * TC:(tg + 1) * TC, :].T.reshape(NDT, 128, NTT, TT)
               .transpose(2, 0, 1, 3)).astype(host_dt)
           for tg in range(TG)]
    upts = [np.ascontiguousarray(up[eg * EC:(eg + 1) * EC, :].T).astype(host_dt)
            .reshape(NDT, 128, EC) for eg in range(EG)]
    dnts = [np.ascontiguousarray(dn[:, eg * EC:(eg + 1) * EC].T).astype(host_dt)
            .reshape(NET, 128, D) for eg in range(EG)]

    in_maps = []
    for c in range(8):
        tg, eg = c // EG, c % EG
        in_maps.append({"xt": xts[tg], "upw": upts[eg], "dwn": dnts[eg]})

    res = run_bass_kernel_spmd(nc, in_maps, list(range(8)))
    LAST_RESULTS = res

    out = np.empty((T, D), dtype=np.float32)
    for tg in range(TG):
        part = (np.asarray(res.results[tg * EG]["ytp"], dtype=np.float32)
                + np.asarray(res.results[tg * EG + 1]["ytp"], dtype=np.float32))
        out[tg * TC:(tg + 1) * TC, :] = part.T
    return out.reshape(B, S, D)



# revision 19
# speedup vs baseline: 1.2351x; 1.2091x over previous
"""MoE (single shared expert) kernel for 8 trn2 NeuronCores.

Math: the reference's top-2 gating over 64 "experts" feeds a single shared
FFN, and the renormalized top-2 weights sum to s/(s+1e-9) with s >= 1/64,
i.e. 1 up to <= 6.4e-8 relative -- below f32 rounding noise.  The whole
module therefore reduces to:  out = silu(x @ up_w.T) @ down_w.T.

Sharding (8 cores): 2D = 4 token-groups x 2 expert-halves.
Each core (tg, eg) computes the partial
    ytp = ( silu(X[tg] @ up_w[eg].T) @ down_w[:, eg].T ).T      [D, TC]
with X[tg] = 2048 tokens, eg = half of the 2048 expert dims.  The host
sums the two partials of each token group and transposes back.

Default mode is bf16 end-to-end (operands, h, and output partials): the
2e-2 absmax tolerance leaves ~5x margin at bf16's ~4e-3, PE rate is the
same 1 cycle/row as f32r, and every DMA byte count halves.  MOE_MM_DTYPE
selects f32r / f32 for the exact paths.
"""

import os
import sys

import numpy as np

for _p in ("/opt/trn_rl_repo",):
    if os.path.isdir(_p) and _p not in sys.path:
        sys.path.insert(0, _p)

import concourse.bass as bass
import concourse.mybir as mybir
import concourse.tile as tile

F32 = mybir.dt.float32
F32R = mybir.dt.float32r
BF16 = mybir.dt.bfloat16


def _ensure_axon_hooks_shim():
    """bass_utils' trace path imports antenv.axon_hooks, which this image
    lacks; give it a no-op hook module so BASS_TRACE=1 degrades gracefully."""
    import types
    if "antenv.axon_hooks" in sys.modules:
        return
    try:
        import antenv
    except ImportError:
        return
    if hasattr(antenv, "axon_hooks"):
        return
    ah = types.ModuleType("antenv.axon_hooks")
    ah._hook = None
    ah.set_axon_ntff_profile_hook = lambda h: setattr(ah, "_hook", h)
    ah.get_axon_ntff_profile_hook = lambda: ah._hook
    sys.modules["antenv.axon_hooks"] = ah
    antenv.axon_hooks = ah


_ensure_axon_hooks_shim()


def _split_multi_waits(nc):
    """This container's walrus encodes at most ONE sync wait per engine
    instruction ("Too many sync wait commands").  Tile routinely emits
    instructions waiting on several semaphores; hoist the extra waits onto
    single-wait NoOps inserted just before, on the same engine."""
    n = 0
    for f in nc.m.functions:
        for blk in f.blocks:
            insts = blk.instructions
            out = []
            for inst in insts:
                si = inst.sync_info
                waits = list(si.on_wait) if si and si.on_wait else []
                if len(waits) > 1:
                    for w in waits[:-1]:
                        n += 1
                        nop = mybir.InstNoOp(name=f"I-wsplit-{n}", ins=[], outs=[])
                        nop.engine = inst.engine
                        nop.sync_info = mybir.SyncInfo(on_wait=[w], on_update=[])
                        nc.register_instruction(nop)
                        out.append(nop)
                    si.on_wait = [waits[-1]]
                out.append(inst)
            if n:
                insts[:] = out
    return n

def _hoist_early_dmas(nc, n=8):
    """Move the first n wait-free SP DMA pushes from the tile block into the
    main block, before the all-engine barrier: the SP then pushes their
    descriptors ~2us earlier (right after its register init), while the Pool
    engine is still working through const memsets and the barrier dance.
    Safe: their completion semaphores are only waited on inside the tile
    block, and nothing touches the target SBUF tiles before the barrier."""
    blocks = {b.name: b for f in nc.m.functions for b in f.blocks}
    main = blocks.get("main")
    tileb = next((b for name, b in blocks.items()
                  if name.startswith("tile_context") and not name.endswith("_end")),
                 None)
    if main is None or tileb is None:
        return 0
    moved = []
    keep = []
    for inst in tileb.instructions:
        if (len(moved) < n
                and type(inst).__name__ == "InstDMACopy"
                and str(inst.engine) == "EngineType.SP"
                and not (inst.sync_info and inst.sync_info.on_wait)):
            moved.append(inst)
        else:
            keep.append(inst)
    if not moved:
        return 0
    tileb.instructions[:] = keep
    # insert before SP's barrier Drain (first SP non-RegisterMove in main)
    idx = next((i for i, inst in enumerate(main.instructions)
                if str(inst.engine) == "EngineType.SP"
                and type(inst).__name__ != "InstRegisterMove"),
               len(main.instructions))
    main.instructions[idx:idx] = moved
    return len(moved)


# Problem shape (hardcoded per contract)
B, S, D, ED = 4, 2048, 1024, 2048
T = B * S                    # 8192 tokens
TG, EG = 4, 2                # token groups x expert-half groups = 8 cores
TC = T // TG                 # tokens per core      = 2048
EC = ED // EG                # expert dims per core = 1024
TT = 512                     # token tile (matmul free dim)
NTT = TC // TT               # 4 token tiles
NDT = D // 128               # 8 d-tiles (contraction 1 / output rows)
NET = EC // 128              # 8 e-tiles (output rows 1 / contraction 2)

_CACHE = {}
LAST_RESULTS = None          # BassKernelResults of the most recent run


def build_nc(mode: str = "bf16") -> bass.Bass:
    """One-core SPMD program: ytp[D, TC] = (silu(x @ upT) @ dwnT).T partial."""
    mm_dt = {"bf16": BF16, "f32r": F32R, "f32": F32}[mode]
    out_dt = BF16 if mode == "bf16" else F32

    nc = bass.Bass()
    # 3D views of the per-core shards so multi-row-block loads batch into a
    # single DMA_DIRECT descriptor push (the SP pushes descriptors at only
    # ~0.6us each; 32 individual pushes paced the whole input stream).
    xt = nc.dram_tensor("xt", [NTT, NDT, 128, TT], mm_dt, kind="ExternalInput")
    upw = nc.dram_tensor("upw", [NDT, 128, EC], mm_dt, kind="ExternalInput")
    dwn = nc.dram_tensor("dwn", [NET, 128, D], mm_dt, kind="ExternalInput")
    ytp = nc.dram_tensor("ytp", [D, TC], out_dt, kind="ExternalOutput")

    with tile.TileContext(nc) as tc:
        with (
            tc.tile_pool(name="wpool", bufs=1) as wpool,
            tc.tile_pool(name="xpool", bufs=8) as xpool,
            tc.tile_pool(name="hpool", bufs=16) as hpool,
            tc.tile_pool(name="ypool", bufs=6) as ypool,
            tc.tile_pool(name="psum", bufs=8, space="PSUM") as psum,
        ):
            # loop1(0) tiles are individual so each (up[di], x0[di]) DMA
            # pair incrementally unblocks 8 matmuls (~0.4MB granularity,
            # keeps the cold di-major sweep fed with zero PE idle -- any
            # >3.4us idle makes the HAM clock-gate re-throttle, measured).
            # Later tensors are one batched DMA each: a single ~0.6us SP
            # descriptor push per tensor, arriving long before their use.
            up_lo = [wpool.tile([128, EC], mm_dt, tag=f"up{di}", name=f"up{di}")
                     for di in range(NDT)]
            x0_lo = [xpool.tile([128, TT], mm_dt, tag="x", name=f"x0_{di}")
                     for di in range(NDT)]
            xb = {tt: wpool.tile([128, NDT * TT], mm_dt, tag=f"xb{tt}",
                                 name=f"xb{tt}")
                  for tt in (2, 3)}
            xb1h = [wpool.tile([128, 4 * TT], mm_dt, tag=f"xb1{i}",
                               name=f"xb1{i}") for i in (0, 1)]
            dn_big = wpool.tile([128, NET * D], mm_dt, tag="dnb", name="dnb")

            def up_ap(di, eb):
                return up_lo[di][:, eb * 128:(eb + 1) * 128]

            def x_ap(tt, di):
                if tt == 0:
                    return x0_lo[di][:]
                if tt == 1:
                    return xb1h[di // 4][:, (di % 4) * TT:(di % 4 + 1) * TT]
                return xb[tt][:, di * TT:(di + 1) * TT]

            def dn_ap(ei, db):
                return dn_big[:, ei * D + db * 128:ei * D + (db + 1) * 128]

            # Warm the PE (HAM clock gate) with dummy matmuls on memset
            # tiles while the initial DMAs stream: the array starts at
            # 1.2GHz and flips to 2.4GHz only after a full ~3.4us activity
            # window of sustained busy-ness; any PE idle before the flip
            # defers it by another window.
            # No memset: the DVE is stuck in the engine preamble until well
            # after the Tensor engine frees up, so a memset dependency makes
            # the warm matmuls start ~2.5us late (measured).  Reading
            # uninitialized SBUF is fine here -- the product is discarded.
            n_warm = int(os.environ.get("MOE_WARM_MM", "7"))
            if n_warm:
                wz = wpool.tile([128, 128], mm_dt, tag="warmw")
                xz = xpool.tile([128, TT], mm_dt, tag="warmx", bufs=1)
                wps = psum.tile([128, TT], F32, tag="ps", name="warm_ps")
                for _ in range(n_warm):
                    nc.tensor.matmul(wps[:], wz[:], xz[:], start=True, stop=True)
                # Writers AFTER the reads: allocates the tiles while leaving
                # the matmuls dependency-free (WAR only orders the memsets).
                nc.vector.memset(wz[:], 0.0)
                nc.vector.memset(xz[:], 0.0)
                wsink = ypool.tile([128, TT], F32, tag="warmy", name="warm_sink")
                nc.vector.tensor_copy(wsink[:], wps[:])

            # All input DMAs up front, in consumption order.  The first four
            # (up[di], x0[di]) pairs arrive one by one and unblock the
            # di-major cold sweep after ~0.4MB; the batched rest needs no
            # further SP pushes, freeing the queue for output DMAs.
            for di in range(NDT):
                nc.sync.dma_start(out=up_lo[di][:], in_=upw[di])
                nc.sync.dma_start(out=x0_lo[di][:], in_=xt[0, di])
            for i in (0, 1):
                nc.sync.dma_start(
                    out=xb1h[i][:].rearrange("p (a c) -> p a c", a=4),
                    in_=xt[1, 4 * i:4 * i + 4].rearrange("a p c -> p a c"),
                )
            nc.sync.dma_start(
                out=dn_big[:].rearrange("p (a c) -> p a c", a=NET),
                in_=dwn[:].rearrange("a p c -> p a c"),
            )
            nc.sync.dma_start(
                out=xb[2][:].rearrange("p (a c) -> p a c", a=NDT),
                in_=xt[2].rearrange("a p c -> p a c"),
            )
            nc.sync.dma_start(
                out=xb[3][:].rearrange("p (a c) -> p a c", a=NDT),
                in_=xt[3].rearrange("a p c -> p a c"),
            )

            hs_all = {}

            def loop1(tt, sweeps):
                # Contraction split into sweeps: all but the last are
                # "init" accumulation passes (eb-inner) needing only their
                # di-range's x/up tiles resident; the last sweep carries the
                # stops, staggered eb-by-eb so the ACT/MUL drain pipelines.
                pss = [psum.tile([128, TT], F32, tag="ps", name=f"ps1_{tt}_{eb}")
                       for eb in range(NET)]
                *init_sweeps, last_sweep = sweeps
                for dis in init_sweeps:
                    for eb in range(NET):
                        for di in dis:
                            nc.tensor.matmul(
                                pss[eb][:], up_ap(di, eb), x_ap(tt, di),
                                start=(di == 0), stop=False,
                            )
                hs = []
                for eb in range(NET):
                    for di in last_sweep:
                        nc.tensor.matmul(
                            pss[eb][:], up_ap(di, eb), x_ap(tt, di),
                            start=(di == 0), stop=(di == NDT - 1),
                        )
                    sg = hpool.tile([128, TT], F32, tag="sg", bufs=3)
                    nc.scalar.activation(
                        sg[:], pss[eb][:], mybir.ActivationFunctionType.Sigmoid
                    )
                    h = hpool.tile([128, TT], mm_dt, tag="h", bufs=20)
                    nc.vector.tensor_mul(h[:], pss[eb][:], sg[:])
                    hs.append(h)
                hs_all[tt] = hs

            def loop2(tt, ei_major):
                t0 = tt * TT
                hs = hs_all.pop(tt)
                if ei_major:
                    ps2s = [psum.tile([128, TT], F32, tag="ps", name=f"ps2_{tt}_{db}")
                            for db in range(NDT)]
                    for ei in range(NET):
                        for db in range(NDT):
                            nc.tensor.matmul(
                                ps2s[db][:], dn_ap(ei, db), hs[ei][:],
                                start=(ei == 0), stop=(ei == NET - 1),
                            )
                    for db in range(NDT):
                        y = ypool.tile([128, TT], out_dt, tag="y")
                        nc.vector.tensor_copy(y[:], ps2s[db][:])
                        nc.sync.dma_start(
                            out=ytp[db * 128:(db + 1) * 128, t0:t0 + TT],
                            in_=y[:],
                        )
                else:
                    for db in range(NDT):
                        if tt == NTT - 1 and db == NDT - 1:
                            # Last group of the kernel: asymmetric 384/128
                            # column split so the final MM->copy->DMA->
                            # completion chain hangs off only a 32KB piece.
                            dsl = slice(db * 128, (db + 1) * 128)
                            wA = 384
                            psA = psum.tile([128, wA], F32, tag="ps",
                                            name="ps2_last_a")
                            psB = psum.tile([128, TT - wA], F32, tag="ps",
                                            name="ps2_last_b")
                            for ei in range(NET):
                                nc.tensor.matmul(
                                    psA[:], dn_ap(ei, db), hs[ei][:, 0:wA],
                                    start=(ei == 0), stop=(ei == NET - 1),
                                )
                            yA = ypool.tile([128, wA], out_dt, tag="y2", bufs=2)
                            nc.vector.tensor_copy(yA[:], psA[:])
                            nc.sync.dma_start(
                                out=ytp[dsl, t0:t0 + wA], in_=yA[:],
                            )
                            for ei in range(NET):
                                nc.tensor.matmul(
                                    psB[:], dn_ap(ei, db), hs[ei][:, wA:TT],
                                    start=(ei == 0), stop=(ei == NET - 1),
                                )
                            yB = ypool.tile([128, TT - wA], out_dt, tag="y2",
                                            bufs=2)
                            nc.vector.tensor_copy(yB[:], psB[:])
                            nc.sync.dma_start(
                                out=ytp[dsl, t0 + wA:t0 + TT], in_=yB[:],
                            )
                            continue
                        ps2 = psum.tile([128, TT], F32, tag="ps",
                                        name=f"ps2_{tt}_{db}")
                        for ei in range(NET):
                            nc.tensor.matmul(
                                ps2[:], dn_ap(ei, db), hs[ei][:],
                                start=(ei == 0), stop=(ei == NET - 1),
                            )
                        y = ypool.tile([128, TT], out_dt, tag="y")
                        nc.vector.tensor_copy(y[:], ps2[:])
                        nc.sync.dma_start(
                            out=ytp[db * 128:(db + 1) * 128, t0:t0 + TT],
                            in_=y[:],
                        )

            loop1(0, [[di] for di in range(NDT)])
            loop1(1, [list(range(4)), list(range(4, NDT))])
            loop2(0, ei_major=True)
            loop1(2, [list(range(NDT))])
            loop2(1, ei_major=False)
            loop1(3, [list(range(NDT))])
            loop2(2, ei_major=False)
            loop2(3, ei_major=False)

    _split_multi_waits(nc)
    if os.environ.get("MOE_EARLY_DMA", "0") == "1":
        _hoist_early_dmas(nc, n=int(os.environ.get("MOE_EARLY_DMA_N", "8")))
    nc.finalize()
    return nc


def _get_nc(mode: str) -> bass.Bass:
    if mode not in _CACHE:
        _CACHE[mode] = build_nc(mode)
    return _CACHE[mode]


def kernel(x, gate_w, up_w, down_w):
    global LAST_RESULTS
    from concourse.bass_utils import run_bass_kernel_spmd

    mode = os.environ.get("MOE_MM_DTYPE", "bf16")
    nc = _get_nc(mode)

    if mode == "bf16":
        import ml_dtypes
        host_dt = np.dtype(ml_dtypes.bfloat16)
    else:
        host_dt = np.dtype(np.float32)

    xf = np.ascontiguousarray(np.asarray(x, dtype=np.float32).reshape(T, D))
    up = np.asarray(up_w, dtype=np.float32)
    dn = np.asarray(down_w, dtype=np.float32)

    # [D, TC] -> [NTT, NDT, 128, TT]: every (tt, di) x-tile is a fully
    # contiguous 128KB block, so DMA descriptors hit max efficiency.
    xts = [np.ascontiguousarray(
               xf[tg * TC:(tg + 1) * TC, :].T.reshape(NDT, 128, NTT, TT)
               .transpose(2, 0, 1, 3)).astype(host_dt)
           for tg in range(TG)]
    upts = [np.ascontiguousarray(up[eg * EC:(eg + 1) * EC, :].T).astype(host_dt)
            .reshape(NDT, 128, EC) for eg in range(EG)]
    dnts = [np.ascontiguousarray(dn[:, eg * EC:(eg + 1) * EC].T).astype(host_dt)
            .reshape(NET, 128, D) for eg in range(EG)]

    in_maps = []
    for c in range(8):
        tg, eg = c // EG, c % EG
        in_maps.append({"xt": xts[tg], "upw": upts[eg], "dwn": dnts[eg]})

    res = run_bass_kernel_spmd(nc, in_maps, list(range(8)))
    LAST_RESULTS = res

    out = np.empty((T, D), dtype=np.float32)
    for tg in range(TG):
        part = (np.asarray(res.results[tg * EG]["ytp"], dtype=np.float32)
                + np.asarray(res.results[tg * EG + 1]["ytp"], dtype=np.float32))
        out[tg * TC:(tg + 1) * TC, :] = part.T
    return out.reshape(B, S, D)

